# revision 22
# baseline (speedup 1.0000x reference)
"""Trainium2 Bass kernel for BinaryCE + rejection-softmax loss.

Reference computation (B=256, C=500, D=256):
    y = labels.astype(f32)                                   # [B, C]
    bce[b] = sum_c( softplus(logits) - y*logits )            # log-sigmoid BCE
    max_sim[b, c] = max_d wf[c, b, d]
    rej[b] = sum_c (labels==0) * relu(sigmoid(max_sim) - 0.3)
    out[b] = bce[b] + rej[b]

Sharding: data-parallel over B across 8 cores (wf on axis 1,
logits/labels on axis 0). Per core: logits [32,500], wf [500,32,256],
labels [32,500] -> out [32]. No cross-device reduction.

Layout: wf is zero-padded to 512 classes on the host and viewed as
[128 partitions, 32768]: partition p holds the 4 consecutive classes
c = 4p..4p+3, so each partition reads one fully contiguous 128 KB
run. 128-partition transfers are mandatory: a [125, N] DMA falls off
the descriptor fast path (half-rate packets plus ~1700 stray 4-byte
packets; whole stream dropped to ~160 GB/s measured).

Structure (all trace-verified on HW):
  * wf streams on the single SWDGE queue in ~1 MB [128, 2048] chunks,
    all descriptor generation front-loaded on the Q7, each chunk in
    its OWN tile (unique name per tile: same-named tiles in a bufs=1
    pool share one slot and serialize the entire stream). A HWDGE
    head chunk was tried and removed: strict ring priority lets it
    block the SWDGE queue while delivering at only ~310 GB/s vs ~420
    sustained on q0 - a wash at best.
  * reduce_max runs 1 elem/cycle/lane on the DVE - no faster mode
    exists on this HW for TensorReduce (fp16 in/out measured
    identical; cast-during-DMA measured 4x slower stream) - which is
    ~1.2x the stream rate, so ~1 MB chunks keep the DVE paced with
    arrival; the tail tapers to 2 x 0.5 MB so only a 1024-elem reduce
    trails the final byte (DMA-completion sem adds ~1.1us before the
    reduce can start).
  * per-group msim tiles: one [128, nb] tile per rejection-chain
    group. A single shared msim tile makes every sigmoid wait on ALL
    reduces (coarse per-tile deps) and pushes every chain past the
    end of the stream.
  * rejection chains (sigmoid -> relu(x-0.3) -> *mask -> ones-matmul
    into PSUM [1,32]) run per half-slab during the stream with their
    elementwise part on the idle gpsimd; the final 4-column chain
    compresses to sig + two fused DVE ops (max((sig-0.3)*mask, 0))
    and carries the accumulation stop flag.
BCE (softplus via exp/ln on ACT) and the label-mask PE transposes run
entirely under the stream; the BCE column is injected into the PSUM
accumulator via an identity-matmul transpose.

Budget per run (fast-HBM runs, ~57.4us total): ~5.8us runtime
preamble, ~2.6us DMA spin-up, ~41us stream at ~400-413 GB/s, ~2.4us
reduce tail, ~1.9us final chain + out-DMA issue, ~2.5us completion
receipt + NEFF epilogue. Run-to-run HW variance is large (57-68us):
slow runs show the stream at ~330 GB/s (HBM contention), everything
else identical.
"""

import sys

for _p in ("/root/.axon_site", "/root/.axon_site/_ro/trn_rl_repo",
           "/root/.axon_site/_ro/pypackages", "/opt/trn_rl_repo"):
    if _p not in sys.path:
        sys.path.append(_p)

import numpy as np

import concourse.bass as bass  # noqa: F401  (registers engine classes)
import concourse.tile as tile
from concourse import bacc, mybir
from concourse.bass_utils import run_bass_kernel_spmd
from concourse.masks import make_identity

F32 = mybir.dt.float32
F16 = mybir.dt.float16
I32 = mybir.dt.int32
AF = mybir.ActivationFunctionType
ALU = mybir.AluOpType
AX = mybir.AxisListType

B, C, D = 256, 500, 256
REJECTION_MARGIN = 0.3
NCORES = 8
BL = B // NCORES          # 32 samples per core
C4 = 4                    # classes per partition
NP = 128                  # partitions; 125-partition DMAs fall off the
                          # fast path (half-rate packets + ~1700 stray
                          # 4B sem packets), so pad classes to 512
CP = NP * C4              # 512 padded classes
SLAB = BL * D             # 8192 elems per (partition, c4)

WF_DT = F32               # fp16 cast-during-DMA measured ~95 GB/s (4x slow); keep f32

# (elem offset, length, c4, first b): ~1MB chunks so each reduce_max
# (DVE, 1 elem/cycle - no faster mode exists for TensorReduce on this
# HW, fp16 included) finishes before the next chunk lands; the tail
# tapers to 2 x 0.5MB so only a 1024-elem reduce and a 4-column chain
# trail the final byte. Everything rides the single SWDGE queue: a
# HWDGE head chunk blocks q0 under strict ring priority while
# delivering at only ~310 GB/s vs q0's ~420 sustained (net loss).
CHUNKS = [
    (0,     2048, 0, 0),
    (2048,  2048, 0, 8),
    (4096,  2048, 0, 16),
    (6144,  2048, 0, 24),
    (8192,  2048, 1, 0),
    (10240, 2048, 1, 8),
    (12288, 2048, 1, 16),
    (14336, 2048, 1, 24),
    (16384, 2048, 2, 0),
    (18432, 2048, 2, 8),
    (20480, 2048, 2, 16),
    (22528, 2048, 2, 24),
    (24576, 2048, 3, 0),
    (26624, 2048, 3, 8),
    (28672, 2048, 3, 16),
    (30720, 1024, 3, 24),
    (31744, 1024, 3, 28),
]
# b-ranges that get their own msim tile + rejection chain. c4 0..2 at
# half-slab granularity (chains overlap the stream); c4=3 per chunk so
# the final dependency cone is one tiny reduce + a 2-column chain.
GROUPS = [
    (0, 0, 16), (0, 16, 16),
    (1, 0, 16), (1, 16, 16),
    (2, 0, 16), (2, 16, 16),
    (3, 0, 8), (3, 8, 8), (3, 16, 8), (3, 24, 4), (3, 28, 4),
]


def build_nc(debug: bool = False):
    nc = bacc.Bacc("TRN2", target_bir_lowering=False, debug=debug)

    logits_d = nc.dram_tensor("logits", [BL, C], F32, kind="ExternalInput")
    wf_d = nc.dram_tensor("wf", [CP, BL, D], F32, kind="ExternalInput")
    labels_d = nc.dram_tensor("labels", [BL, C], I32, kind="ExternalInput")
    out_d = nc.dram_tensor("out", [1, BL], F32, kind="ExternalOutput")

    # [128, 32768]: partition p = classes 4p..4p+3, contiguous per partition
    wfv = wf_d[:].rearrange("(p c4) b d -> p (c4 b d)", c4=C4)

    with tile.TileContext(nc) as tc:
        with (
            tc.tile_pool(name="consts", bufs=1) as consts,
            tc.tile_pool(name="psum_t", bufs=2, space="PSUM") as psum_t,
            tc.tile_pool(name="psum_acc", bufs=1, space="PSUM") as psum_acc,
        ):
            # --- wf stream: all descgens first on the Q7, distinct
            # buffers so nothing ever waits on compute ------------------
            wfts = []
            for i, (off, ln, _c4, _b0) in enumerate(CHUNKS):
                # unique name per chunk: the tile tag defaults to the
                # assignee name, and same-tag tiles in a bufs=1 pool
                # share ONE slot (serializes the whole stream).
                wft = consts.tile([NP, ln], WF_DT, name=f"wft{i}")
                nc.gpsimd.dma_start(wft[:], wfv[:, off:off + ln])
                wfts.append(wft)

            # --- small inputs on the sync ring (tiny, independent) ------
            logits_sb = consts.tile([BL, C], F32)
            nc.sync.dma_start(logits_sb[:], logits_d[:])
            labels_sb = consts.tile([BL, C], I32)
            nc.sync.dma_start(labels_sb[:], labels_d[:])

            # identity after the descgens: gpsimd program order would
            # otherwise delay the first wf chunk by the Q7 launches.
            ident = consts.tile([BL, BL], F32)
            make_identity(nc, ident[:])

            labels_f = consts.tile([BL, C], F32)
            nc.vector.tensor_copy(labels_f[:], labels_sb[:])

            ones = consts.tile([NP, 1], F32)
            nc.vector.memset(ones[:], 1.0)

            # --- BCE part in natural [b, c] layout -------------------------
            # softplus(x) = ln(exp(x) + 1); no Softplus LUT on TRN2.
            # Safe: |logits| <~ 5 so exp() cannot overflow.
            exp_tmp = consts.tile([BL, C], F32)
            nc.scalar.activation(exp_tmp[:], logits_sb[:], AF.Exp)
            sp_tmp = consts.tile([BL, C], F32)
            sp_sum = consts.tile([BL, 1], F32)
            nc.scalar.activation(sp_tmp[:], exp_tmp[:], AF.Ln, bias=1.0,
                                 accum_out=sp_sum[:])
            yx_tmp = consts.tile([BL, C], F32)
            yx_sum = consts.tile([BL, 1], F32)
            nc.vector.tensor_mul(yx_tmp[:], labels_f[:], logits_sb[:])
            nc.vector.reduce_sum(yx_sum[:], yx_tmp[:], axis=AX.X)
            bce_col = consts.tile([BL, 1], F32)
            nc.vector.tensor_sub(bce_col[:], sp_sum[:], yx_sum[:])

            # --- mask = 1 - labels^T in [p, c4, b] layout (c = 4p + c4) ----
            # Padded classes c >= 500 keep mask 0 from the memset, so the
            # zero-padded wf rows contribute nothing.
            mask_sb = consts.tile([NP, C4, BL], F32)
            nc.vector.memset(mask_sb[:], 0.0)
            for c4 in range(C4):
                labT = psum_t.tile([C // C4, BL], F32, tag="labT")
                nc.tensor.matmul(labT[:], labels_f[:, c4::C4], ident[:],
                                 start=True, stop=True)
                nc.scalar.activation(mask_sb[:C // C4, c4, :], labT[:],
                                     AF.Identity, bias=1.0, scale=-1.0)

            # --- PSUM accumulator [1, 32]; BCE row first -------------------
            acc = psum_acc.tile([1, BL], F32)
            nc.tensor.matmul(acc[:], bce_col[:], ident[:],
                             start=True, stop=False)

            # --- stream reduces + masked rejection chains ------------------
            # One msim tile per GROUP: with a single [NP, C4, BL] tile
            # the dep tracker makes EVERY sigmoid wait for ALL reduces
            # (coarse per-tile deps), pushing every chain past the end
            # of the stream (trace-verified on the baseline).
            group_of = {}              # b index -> group key
            msim_t = {}
            for (c4, g0, gn) in GROUPS:
                msim_t[(c4, g0)] = consts.tile([NP, gn], WF_DT,
                                               name=f"msim{c4}_{g0}")
                for b in range(g0, g0 + gn):
                    group_of[(c4, b)] = (c4, g0, gn)

            def red(chunk_ap, c4, b0, nb):
                c4g, g0, gn = group_of[(c4, b0)]
                o = b0 - g0
                nc.vector.reduce_max(
                    msim_t[(c4, g0)][:, o:o + nb],
                    chunk_ap.rearrange("p (b d) -> p b d", d=D), axis=AX.X)

            neg_margin = consts.tile([NP, 1], F32)
            nc.vector.memset(neg_margin[:], -REJECTION_MARGIN)

            def chain(c4, g0, gn, stop, mul_eng):
                # early chains run their elementwise part on gpsimd
                # (idle after descgen) so the DVE queue stays pure
                # reduces. The final chain compresses to sig -> two
                # fused DVE ops: rejm = max((sig - 0.3) * mask, 0) ==
                # relu(sig - 0.3) * mask since mask is 0/1. (The fused
                # TensorScalarPtr form doesn't exist on Pool, so the
                # gpsimd path keeps the ACT relu.)
                sl = slice(g0, g0 + gn)
                sig = consts.tile([NP, gn], F32, name=f"sig{c4}_{g0}")
                nc.scalar.activation(sig[:], msim_t[(c4, g0)][:], AF.Sigmoid)
                rejm = consts.tile([NP, gn], F32, name=f"rejm{c4}_{g0}")
                if mul_eng is nc.vector:
                    rej = consts.tile([NP, gn], F32, name=f"rej{c4}_{g0}")
                    mul_eng.scalar_tensor_tensor(
                        rej[:], sig[:], REJECTION_MARGIN, mask_sb[:, c4, sl],
                        op0=ALU.subtract, op1=ALU.mult)
                    mul_eng.tensor_scalar_max(rejm[:], rej[:], 0.0)
                else:
                    rej = consts.tile([NP, gn], F32, name=f"rej{c4}_{g0}")
                    nc.scalar.activation(rej[:], sig[:], AF.Relu,
                                         bias=neg_margin[:])
                    mul_eng.tensor_mul(rejm[:], rej[:], mask_sb[:, c4, sl])
                nc.tensor.matmul(acc[:, sl], ones[:], rejm[:],
                                 start=False, stop=stop)

            covered = {k: 0 for k in msim_t}
            n_chunks = len(CHUNKS)
            for i, (off, ln, c4, b0) in enumerate(CHUNKS):
                nb = ln // D
                red(wfts[i][:], c4, b0, nb)
                c4g, g0, gn = group_of[(c4, b0)]
                covered[(c4, g0)] += nb
                if covered[(c4, g0)] == gn:
                    is_last = (i == n_chunks - 1)
                    mul_eng = nc.vector if is_last else nc.gpsimd
                    chain(c4, g0, gn, stop=(c4 == C4 - 1), mul_eng=mul_eng)

            out_sb = consts.tile([1, BL], F32)
            nc.vector.tensor_copy(out_sb[:], acc[:])
            nc.scalar.dma_start(out_d[:], out_sb[:])

    nc.compile()
    return nc


_NC_CACHE = None


def _get_nc():
    global _NC_CACHE
    if _NC_CACHE is None:
        _NC_CACHE = build_nc()
    return _NC_CACHE


def _in_maps(logits, wf, labels):
    maps = []
    for k in range(NCORES):
        b0 = k * BL
        wf_pad = np.zeros((CP, BL, D), dtype=np.float32)
        wf_pad[:C] = wf[:, b0:b0 + BL, :]
        maps.append({
            "logits": np.ascontiguousarray(logits[b0:b0 + BL]),
            "wf": wf_pad,
            "labels": np.ascontiguousarray(labels[b0:b0 + BL]),
        })
    return maps


def run(logits, wf, labels, trace: bool = False, tmpdir: str | None = None):
    """Run on all 8 cores; returns (full_output [B], BassKernelResults)."""
    logits = np.asarray(logits, dtype=np.float32)
    wf = np.asarray(wf, dtype=np.float32)
    labels = np.asarray(labels, dtype=np.int32)
    assert logits.shape == (B, C) and wf.shape == (C, B, D) \
        and labels.shape == (B, C)

    nc = _get_nc()
    res = run_bass_kernel_spmd(nc, _in_maps(logits, wf, labels),
                               list(range(NCORES)), trace=trace,
                               tmpdir=tmpdir)
    out = np.concatenate(
        [np.asarray(res.results[k]["out"]).reshape(BL) for k in range(NCORES)])
    return out.astype(np.float32), res


def kernel(logits, wf, labels):
    out, _ = run(logits, wf, labels)
    return out
